# revision 1
# baseline (speedup 1.0000x reference)
"""Trainium2 Bass kernel: float32 -> 32-channel bit-plane encoding.

For input x [4096, 512] f32, produces out [4096, 512, 32] f32 where
out[b, f, 0] = (x[b,f] < 0) and out[b, f, 1+j] = bit (30-j) of
bitcast_int32(|x[b,f]|), MSB first.

Host-side repack makes every channel a uniform positive-mask bit test:
  i' = (bitcast_i32(x) & 0x7FFFFFFF) | ((x < 0) << 31)
so channel k is Sign(uint32(i' & mask[k])) with mask[0] = 0x80000000 and
mask[k] = 1 << (31-k).  (bits 30..0 of x equal those of |x|, and replacing
bit 31 with the float compare keeps -0.0 / NaN semantics exact.)

Sharded row-wise over 8 NeuronCores (512 rows each).  Per core:
  pass1 (VectorE):  and_t[p, f, k] = i'[p,f] & mask[k]   (uint32; masks are
                    packed into the input's first 32 columns so one DMA feeds
                    both operands)
  pass2 (ScalarE):  out = Sign(and_t)  (uint32 -> f32: {0, 2^s} -> {0.0, 1.0})
  out-DMA via HWDGE (sync engine) in large contiguous pieces.

Compute granularity (128-col chunks) is finer than DMA granularity (256-col
pieces): the out-DMA stream is the bottleneck (~32MB/core at ~450GB/s), so
pieces are few and large, while fine compute chunks hand bytes to the DMA
stream as early as possible.  Small leading chunks collapse the ramp.
"""

import sys

if "/opt/trn_rl_repo" not in sys.path:
    sys.path.insert(0, "/opt/trn_rl_repo")

import numpy as np

import concourse.bass as bass
import concourse.mybir as mybir

P = 128          # SBUF partitions
F = 512          # features per row
K = 32           # output channels per feature
N_CORES = 8
ROWS_TOTAL = 4096
ROWS = ROWS_TOTAL // N_CORES   # rows per core
NRT = ROWS // P                # row tiles per core (4)
XW = K + F                     # packed input width (32 mask cols + x columns)
FIRST_COLS = K + 64            # first in-DMA slice: masks + first 64 x cols
FCH_MAX = 256                  # max DMA piece width (columns)

# (chunks, pieces) per row block.  Chunks drive TT/Sign; pieces drive the
# out-DMA.  Piece boundaries must align with chunk boundaries.
SCHED_RB0 = ([32, 32, 64, 128, 128, 128], [32, 32, 64, 128, 128, 128])
SCHED_RB = ([128, 128, 128, 128], [256, 256])

NBUF_AT = 2     # at buffers (chunk-sized)
NBUF_OT = 4     # ot buffers (piece-sized)


def _masks_np() -> np.ndarray:
    vals = [1 << (31 - k) for k in range(K)]   # k=0 -> 0x80000000
    return np.array(vals, dtype=np.int64).astype(np.uint32).view(np.int32)


def _schedule():
    """Build (chunks, pieces) lists.

    chunk: (ci, rt, c_off, c_len, piece_index)
    piece: (pi, rt, c_off, c_len, last_chunk_index)
    """
    chunks, pieces = [], []
    for rt in range(NRT):
        ch_list, pc_list = SCHED_RB0 if rt == 0 else SCHED_RB
        assert sum(ch_list) == F and sum(pc_list) == F
        # map chunk offsets to piece indices
        pc_bounds = []
        off = 0
        for pl in pc_list:
            pc_bounds.append((off, off + pl))
            off += pl
        pc_base = len(pieces)
        for j, (a, b) in enumerate(pc_bounds):
            pieces.append([pc_base + j, rt, a, b - a, -1])
        off = 0
        for cl in ch_list:
            pj = next(j for j, (a, b) in enumerate(pc_bounds)
                      if a <= off and off + cl <= b)
            ci = len(chunks)
            chunks.append((ci, rt, off, cl, pc_base + pj))
            pieces[pc_base + pj][4] = ci
            off += cl
    return chunks, [tuple(p) for p in pieces]


def build_nc(in_dma="sp", warm_act=True) -> bass.Bass:
    nc = bass.Bass("TRN2", target_bir_lowering=False, debug=False)
    i32, f32, u32 = mybir.dt.int32, mybir.dt.float32, mybir.dt.uint32

    xm = nc.declare_dram_parameter("xm", [ROWS, XW], i32, isOutput=False)
    out = nc.declare_dram_parameter("out", [ROWS, F * K], f32, isOutput=True)
    xm_ap, out_ap = xm.ap(), out.ap()

    chunks, pieces = _schedule()
    # per-piece: how many times its ot slot was used before (for WAR waits)
    slot_use = {}
    piece_slot_prev = {}
    for pi, rt, c_off, c_len, lc in pieces:
        s = pi % NBUF_OT
        piece_slot_prev[pi] = slot_use.get(s, 0)
        slot_use[s] = piece_slot_prev[pi] + 1
    # piece offset within its ot slot: piece's own c_off relative to piece
    # start is 0; chunks write at (chunk.c_off - piece.c_off) * K

    from contextlib import ExitStack
    with ExitStack() as ctx:
        xt = [ctx.enter_context(nc.sbuf_tensor(f"xt{b}", [P, XW], i32))
              for b in range(NRT)]
        at = [ctx.enter_context(nc.sbuf_tensor(f"at{b}", [P, 128 * K], u32))
              for b in range(NBUF_AT)]
        ot = [ctx.enter_context(nc.sbuf_tensor(f"ot{b}", [P, FCH_MAX * K], f32))
              for b in range(NBUF_OT)]
        warm = ctx.enter_context(nc.sbuf_tensor("warm", [P, 1], f32))

        in_sem = [ctx.enter_context(nc.semaphore(f"in_sem{b}"))
                  for b in range(NRT)]
        in0a_sem = ctx.enter_context(nc.semaphore("in0a_sem"))
        od_sem = [ctx.enter_context(nc.semaphore(f"od_sem{b}"))
                  for b in range(NBUF_OT)]
        tt_sem = ctx.enter_context(nc.semaphore("tt_sem"))
        act_sem = ctx.enter_context(nc.semaphore("act_sem"))

        ctx.enter_context(nc.Block())
        block = nc.cur_block

        @block.vector
        def _(vec: bass.BassEngine):
            seen_rb = -1
            for ci, rt, c_off, c_len, pi in chunks:
                if rt == 0:
                    if ci == 0:
                        vec.wait_ge(in0a_sem, 16)
                    elif c_off + c_len > FIRST_COLS - K and seen_rb < 0:
                        vec.wait_ge(in_sem[0], 16)
                        seen_rb = 0
                elif rt != seen_rb:
                    vec.wait_ge(in_sem[rt], 16)
                    seen_rb = rt
                if ci >= NBUF_AT:
                    # at[ci%NBUF_AT] is free once Sign(ci-NBUF_AT) read it
                    vec.wait_ge(act_sem, ci - NBUF_AT + 1)
                in0 = xt[rt][:, K + c_off:K + c_off + c_len].bitcast(u32) \
                    .unsqueeze(-1).broadcast_to([P, c_len, K])
                in1 = xt[rt][:, 0:K].bitcast(u32) \
                    .unsqueeze(1).broadcast_to([P, c_len, K])
                o3 = at[ci % NBUF_AT][:, 0:c_len * K] \
                    .rearrange("p (f k) -> p f k", k=K)
                vec.tensor_tensor(
                    o3, in0, in1, mybir.AluOpType.bitwise_and
                ).then_inc(tt_sem)

        @block.scalar
        def _(sc: bass.BassEngine):
            if warm_act:
                # scale=0 -> input is not read (safe on uninitialized SBUF)
                sc.activation(warm[:], warm[:],
                              mybir.ActivationFunctionType.Sign, scale=0.0)
            seen_piece = -1
            for ci, rt, c_off, c_len, pi in chunks:
                sc.wait_ge(tt_sem, ci + 1)
                if pi != seen_piece:
                    # first chunk of a piece: its ot slot must be drained
                    prev = piece_slot_prev[pi]
                    if prev > 0:
                        sc.wait_ge(od_sem[pi % NBUF_OT], 16 * prev)
                    seen_piece = pi
                p_off = c_off - pieces[pi][2]
                sc.activation(
                    ot[pi % NBUF_OT][:, p_off * K:(p_off + c_len) * K],
                    at[ci % NBUF_AT][:, 0:c_len * K],
                    mybir.ActivationFunctionType.Sign,
                ).then_inc(act_sem)

        if in_dma == "gp":
            @block.gpsimd
            def _(gp: bass.BassEngine):
                gp.dma_start(
                    xt[0][:, 0:FIRST_COLS], xm_ap[0:P, 0:FIRST_COLS]
                ).then_inc(in0a_sem, 16)
                gp.dma_start(
                    xt[0][:, FIRST_COLS:XW], xm_ap[0:P, FIRST_COLS:XW]
                ).then_inc(in_sem[0], 16)
                for rt in range(1, NRT):
                    gp.dma_start(
                        xt[rt][:], xm_ap[rt * P:(rt + 1) * P, :]
                    ).then_inc(in_sem[rt], 16)

        @block.sync
        def _(sp: bass.BassEngine):
            if in_dma == "sp":
                sp.dma_start(
                    xt[0][:, 0:FIRST_COLS], xm_ap[0:P, 0:FIRST_COLS]
                ).then_inc(in0a_sem, 16)
                sp.dma_start(
                    xt[0][:, FIRST_COLS:XW], xm_ap[0:P, FIRST_COLS:XW]
                ).then_inc(in_sem[0], 16)
                for rt in range(1, NRT):
                    sp.dma_start(
                        xt[rt][:], xm_ap[rt * P:(rt + 1) * P, :]
                    ).then_inc(in_sem[rt], 16)
            for pi, rt, c_off, c_len, lc in pieces:
                sp.wait_ge(act_sem, lc + 1)
                sp.dma_start(
                    out_ap[rt * P:(rt + 1) * P,
                           c_off * K:(c_off + c_len) * K],
                    ot[pi % NBUF_OT][:, 0:c_len * K],
                ).then_inc(od_sem[pi % NBUF_OT], 16)

    return nc


_NC_CACHE = None


def _get_nc():
    global _NC_CACHE
    if _NC_CACHE is None:
        _NC_CACHE = build_nc()
    return _NC_CACHE


def pack_shard(x_shard: np.ndarray) -> np.ndarray:
    """[ROWS, F] f32 -> [ROWS, K+F] int32: the 32 mask columns followed by
    sign-normalized bitcast columns."""
    x_shard = np.ascontiguousarray(x_shard)
    xi = x_shard.view(np.uint32)
    xi = (xi & np.uint32(0x7FFFFFFF)) | \
        ((x_shard < 0).astype(np.uint32) << np.uint32(31))
    m = np.broadcast_to(_masks_np(), (x_shard.shape[0], K))
    return np.ascontiguousarray(
        np.concatenate([m, xi.view(np.int32)], axis=1))


def kernel(x: np.ndarray) -> np.ndarray:
    from concourse.bass_utils import run_bass_kernel_spmd

    x = np.asarray(x, dtype=np.float32)
    assert x.shape == (ROWS_TOTAL, F), x.shape
    nc = _get_nc()
    in_maps = [
        {"xm": pack_shard(x[i * ROWS:(i + 1) * ROWS])} for i in range(N_CORES)
    ]
    res = run_bass_kernel_spmd(nc, in_maps, list(range(N_CORES)))
    parts = [res.results[i]["out"].reshape(ROWS, F, K) for i in range(N_CORES)]
    return np.concatenate(parts, axis=0)



# revision 16
# speedup vs baseline: 2.5811x; 2.5811x over previous
"""Trainium2 Bass kernel: float32 -> 32-channel bit-plane encoding.

For input x [4096, 512] f32, produces out [4096, 512, 32] f32 where
out[b, f, 0] = (x[b,f] < 0) and out[b, f, 1+j] = bit (30-j) of
bitcast_int32(|x[b,f]|), MSB first.

Host-side repack merges the sign test into bit 31:
  i' = (bitcast_u32(x) & 0x7FFFFFFF) | ((x < 0) << 31)
and splits i' into two u16 planes (hi = bits 31..16 -> channels 0..15,
lo = bits 15..0 -> channels 16..31), stored per row as [hi(512), lo(512)].

Device compute is ONE fused DVE tensor_scalar per channel PAIR:
  t = (v >> s) & 0x0101        (u16 -> u16, both ops bitwise-class)
puts bit s in byte 0 and bit s+8 in byte 1 of the u16 lane — two final
u8 output channels per processed element.  With u16 in/out, packed,
SBUF-only operands this runs in the DVE 4x_2p perf mode (0.25 cyc/elem),
so the whole 8.39M-byte/core output costs only ~8.5us of VectorE time.
s in 0..7 over the hi plane covers channel pairs (15-s, 7-s); over the
lo plane (31-s, 23-s).

The device writes uint8 pairs (values exactly 0/1) grouped as 16
channel-pair planes [16, rows, 512] u16; the host reassembles the
[rows, 512, 32] channel order with a fixed 32-wide permutation during
the u8 -> f32 widening.  Writing u8 instead of f32 cuts the out-DMA
stream 4x (8.39 MB/core), which moves the roofline from ~87us to ~23us;
the kernel is out-DMA bound, VectorE is ~50% busy, and the other
engines only issue DMAs (in-DMA rides the gpsimd queue so the sync
queue's pieces start unobstructed).
"""

import sys

if "/opt/trn_rl_repo" not in sys.path:
    sys.path.insert(0, "/opt/trn_rl_repo")

import numpy as np

import concourse.bass as bass
import concourse.mybir as mybir

P = 128          # SBUF partitions
F = 512          # features per row
K = 32           # output channels per feature
NPAIR = 16       # channel-pair planes
N_CORES = 8
ROWS_TOTAL = 4096
ROWS = ROWS_TOTAL // N_CORES   # rows per core
NRT = ROWS // P                # row tiles per core (4)

# out-DMA pieces / DVE sync blocks: (rt, f0, f1).  rt0 quartered so the
# first piece ships early; later tiles are whole (DVE runs far ahead of
# the DMA stream, so only the ramp matters).
BLOCKS = [(0, 0, 128), (0, 128, 256), (0, 256, 384), (0, 384, 512),
          (1, 0, 512), (2, 0, 512), (3, 0, 512)]

# plane j covers: j<8 -> hi plane, s=j, channels (15-j @byte0, 7-j @byte1)
#                 j>=8 -> lo plane, s=j-8, channels (31-s @byte0, 23-s @byte1)
_PLANE_K_SEQ = []
for _j in range(8):
    _PLANE_K_SEQ += [15 - _j, 7 - _j]
for _j in range(8):
    _PLANE_K_SEQ += [31 - _j, 23 - _j]
# PERM[k] = position of channel k in the device byte stream
PERM = np.array([_PLANE_K_SEQ.index(k) for k in range(K)], dtype=np.int64)


def build_nc() -> bass.Bass:
    nc = bass.Bass("TRN2", target_bir_lowering=False, debug=False)
    u16 = mybir.dt.uint16
    SHR, AND = mybir.AluOpType.logical_shift_right, mybir.AluOpType.bitwise_and

    xm = nc.declare_dram_parameter("xm", [ROWS, 2 * F], u16, isOutput=False)
    out = nc.declare_dram_parameter("out", [NPAIR * ROWS, F], u16,
                                    isOutput=True)
    xm_ap, out_ap = xm.ap(), out.ap()
    # [r, q, f] view of out (q = pair plane), iteration-matched to SBUF
    out_rqf = out_ap.rearrange("(q r) f -> r q f", r=ROWS)

    from contextlib import ExitStack
    with ExitStack() as ctx:
        xt = [ctx.enter_context(nc.sbuf_tensor(f"xt{b}", [P, 2 * F], u16))
              for b in range(NRT)]
        po = [ctx.enter_context(nc.sbuf_tensor(f"po{b}", [P, NPAIR * F], u16))
              for b in range(NRT)]

        in_sem = [ctx.enter_context(nc.semaphore(f"in_sem{b}"))
                  for b in range(NRT)]
        vd_sem = ctx.enter_context(nc.semaphore("vd_sem"))
        od_sem = ctx.enter_context(nc.semaphore("od_sem"))

        ctx.enter_context(nc.Block())
        block = nc.cur_block

        @block.gpsimd
        def _(gp: bass.BassEngine):
            for rt in range(NRT):
                gp.dma_start(
                    xt[rt][:], xm_ap[rt * P:(rt + 1) * P, :]
                ).then_inc(in_sem[rt], 16)

        @block.vector
        def _(vec: bass.BassEngine):
            seen_rt = -1
            for rt, f0, f1 in BLOCKS:
                if rt != seen_rt:
                    vec.wait_ge(in_sem[rt], 16)
                    seen_rt = rt
                for j in range(NPAIR):
                    plane, s = (0, j) if j < 8 else (F, j - 8)
                    o = po[rt][:, j * F + f0:j * F + f1]
                    i0 = xt[rt][:, plane + f0:plane + f1]
                    vec.tensor_scalar(o, i0, s, 0x0101, SHR, AND) \
                        .then_inc(vd_sem)

        @block.sync
        def _(sp: bass.BassEngine):
            for b, (rt, f0, f1) in enumerate(BLOCKS):
                sp.wait_ge(vd_sem, NPAIR * (b + 1))
                src = po[rt][:, :] \
                    .rearrange("p (q f) -> p q f", f=F)[:, :, f0:f1]
                sp.dma_start(
                    out_rqf[rt * P:(rt + 1) * P, :, f0:f1],
                    src,
                ).then_inc(od_sem, 16)

    return nc


_NC_CACHE = None


def _get_nc():
    global _NC_CACHE
    if _NC_CACHE is None:
        _NC_CACHE = build_nc()
    return _NC_CACHE


def pack_shard(x_shard: np.ndarray) -> np.ndarray:
    """[ROWS, F] f32 -> [ROWS, 2F] u16: hi plane (bits 31..16, with bit 31
    replaced by the x<0 test) then lo plane (bits 15..0)."""
    x_shard = np.ascontiguousarray(x_shard)
    xi = x_shard.view(np.uint32)
    xi = (xi & np.uint32(0x7FFFFFFF)) | \
        ((x_shard < 0).astype(np.uint32) << np.uint32(31))
    hi = (xi >> np.uint32(16)).astype(np.uint16)
    lo = (xi & np.uint32(0xFFFF)).astype(np.uint16)
    return np.ascontiguousarray(np.concatenate([hi, lo], axis=1))


def unpack_result(out_dev: np.ndarray) -> np.ndarray:
    """Device [NPAIR*ROWS, F] u16 -> [ROWS, F, K] f32 in channel order."""
    raw = out_dev.reshape(NPAIR, ROWS, F).view(np.uint8) \
        .reshape(NPAIR, ROWS, F, 2)
    byte_k = raw.transpose(1, 2, 0, 3).reshape(ROWS, F, K)
    return byte_k[:, :, PERM].astype(np.float32)


def kernel(x: np.ndarray) -> np.ndarray:
    from concourse.bass_utils import run_bass_kernel_spmd

    x = np.asarray(x, dtype=np.float32)
    assert x.shape == (ROWS_TOTAL, F), x.shape
    nc = _get_nc()
    in_maps = [
        {"xm": pack_shard(x[i * ROWS:(i + 1) * ROWS])} for i in range(N_CORES)
    ]
    res = run_bass_kernel_spmd(nc, in_maps, list(range(N_CORES)))
    parts = [unpack_result(res.results[i]["out"]) for i in range(N_CORES)]
    return np.concatenate(parts, axis=0)


# revision 17
# speedup vs baseline: 3.4625x; 1.3415x over previous
"""Trainium2 Bass kernel: float32 -> 32-channel bit-plane encoding.

For input x [4096, 512] f32, produces out [4096, 512, 32] f32 where
out[b, f, 0] = (x[b,f] < 0) and out[b, f, 1+j] = bit (30-j) of
bitcast_int32(|x[b,f]|), MSB first.

Host-side repack merges the sign test into bit 31:
  i' = (bitcast_u32(x) & 0x7FFFFFFF) | ((x < 0) << 31)
and splits i' into two u16 planes (hi = bits 31..16 -> channels 0..15,
lo = bits 15..0 -> channels 16..31), stored per row as [hi(512), lo(512)].

Device compute is ONE fused DVE tensor_scalar per channel PAIR:
  t = (v >> s) & 0x0101        (u16 -> u16, both ops bitwise-class)
puts bit s in byte 0 and bit s+8 in byte 1 of the u16 lane — two final
u8 output channels per processed element.  With u16 in/out, packed,
SBUF-only operands this runs in the DVE 4x_2p perf mode (0.25
cyc/elem), so the whole 8.39M-byte/core output costs ~8.5us of VectorE
time; the kernel is out-DMA bound.  s in 0..7 over the hi plane covers
channel pairs (15-s, 7-s); over the lo plane (31-s, 23-s).

The device writes uint8 pairs (values exactly 0/1) laid out
[rows, 16 pair-planes, 512] u16, so each output row is one contiguous
16KB run in BOTH SBUF and DRAM: out-DMA descriptors stay large (the
~420 GB/s regime measured on this part).  Writing u8 instead of f32
cuts the out-DMA stream 4x (8.39 MB/core), moving the roofline from
~87us to ~23us.  The host reassembles [rows, 512, 32] channel order
with a fixed 32-wide permutation during the u8 -> f32 widening.

Ramp details: in-DMAs ride the sync queue (rt0, hi-plane half first so
VectorE starts after ~128KB) and the scalar queue (rt1-3) — NOT the
gpsimd queue, whose software DGE adds ~3us.  Out pieces are pair-plane
ranges (quarters of rt0 first), each waiting only on the VectorE
instructions that filled it.
"""

import sys

if "/opt/trn_rl_repo" not in sys.path:
    sys.path.insert(0, "/opt/trn_rl_repo")

import numpy as np

import concourse.bass as bass
import concourse.mybir as mybir

P = 128          # SBUF partitions
F = 512          # features per row
K = 32           # output channels per feature
NPAIR = 16       # channel-pair planes
N_CORES = 8
ROWS_TOTAL = 4096
ROWS = ROWS_TOTAL // N_CORES   # rows per core
NRT = ROWS // P                # row tiles per core (4)

# out-DMA pieces: (rt, q0, q1) — pair-plane ranges within a row tile.
PIECES = [(0, 0, 4), (0, 4, 8), (0, 8, 12), (0, 12, 16),
          (1, 0, 8), (1, 8, 16),
          (2, 0, 16),
          (3, 0, 16)]

# plane j covers: j<8 -> hi plane, s=j, channels (15-j @byte0, 7-j @byte1)
#                 j>=8 -> lo plane, s=j-8, channels (31-s @byte0, 23-s @byte1)
_PLANE_K_SEQ = []
for _j in range(8):
    _PLANE_K_SEQ += [15 - _j, 7 - _j]
for _j in range(8):
    _PLANE_K_SEQ += [31 - _j, 23 - _j]
# PERM[k] = position of channel k in the device byte stream of one (row, f)
PERM = np.array([_PLANE_K_SEQ.index(k) for k in range(K)], dtype=np.int64)


def build_nc() -> bass.Bass:
    nc = bass.Bass("TRN2", target_bir_lowering=False, debug=False)
    u16 = mybir.dt.uint16
    SHR, AND = mybir.AluOpType.logical_shift_right, mybir.AluOpType.bitwise_and

    xm = nc.declare_dram_parameter("xm", [ROWS, 2 * F], u16, isOutput=False)
    out = nc.declare_dram_parameter("out", [ROWS * NPAIR, F], u16,
                                    isOutput=True)
    xm_ap, out_ap = xm.ap(), out.ap()
    # [r, q, f] view of out (q = pair plane, innermost block of each row)
    out_rqf = out_ap.rearrange("(r q) f -> r q f", q=NPAIR)

    from contextlib import ExitStack
    with ExitStack() as ctx:
        xt = [ctx.enter_context(nc.sbuf_tensor(f"xt{b}", [P, 2 * F], u16))
              for b in range(NRT)]
        po = [ctx.enter_context(nc.sbuf_tensor(f"po{b}", [P, NPAIR * F], u16))
              for b in range(NRT)]

        in_sem = [ctx.enter_context(nc.semaphore(f"in_sem{b}"))
                  for b in range(NRT)]
        in0b_sem = ctx.enter_context(nc.semaphore("in0b_sem"))
        vd_sem = ctx.enter_context(nc.semaphore("vd_sem"))
        od_sem = ctx.enter_context(nc.semaphore("od_sem"))

        ctx.enter_context(nc.Block())
        block = nc.cur_block

        @block.vector
        def _(vec: bass.BassEngine):
            seen_rt = -1
            for rt in range(NRT):
                for q in range(NPAIR):
                    if rt == 0 and q == 0:
                        vec.wait_ge(in_sem[0], 16)       # rt0 hi plane
                    elif rt == 0 and q == 8:
                        vec.wait_ge(in0b_sem, 16)        # rt0 lo plane
                    elif rt != seen_rt:
                        vec.wait_ge(in_sem[rt], 16)
                        seen_rt = rt
                    plane, s = (0, q) if q < 8 else (F, q - 8)
                    o = po[rt][:, q * F:(q + 1) * F]
                    i0 = xt[rt][:, plane:plane + F]
                    vec.tensor_scalar(o, i0, s, 0x0101, SHR, AND) \
                        .then_inc(vd_sem)

        @block.scalar
        def _(sc: bass.BassEngine):
            for rt in range(1, NRT):
                sc.dma_start(
                    xt[rt][:], xm_ap[rt * P:(rt + 1) * P, :]
                ).then_inc(in_sem[rt], 16)

        @block.sync
        def _(sp: bass.BassEngine):
            # rt0 input: hi plane first so VectorE starts after 128KB
            sp.dma_start(
                xt[0][:, 0:F], xm_ap[0:P, 0:F]
            ).then_inc(in_sem[0], 16)
            sp.dma_start(
                xt[0][:, F:2 * F], xm_ap[0:P, F:2 * F]
            ).then_inc(in0b_sem, 16)
            for rt, q0, q1 in PIECES:
                sp.wait_ge(vd_sem, rt * NPAIR + q1)
                sp.dma_start(
                    out_rqf[rt * P:(rt + 1) * P, q0:q1, :],
                    po[rt][:, q0 * F:q1 * F]
                    .rearrange("p (q f) -> p q f", f=F),
                ).then_inc(od_sem, 16)

    return nc


_NC_CACHE = None


def _get_nc():
    global _NC_CACHE
    if _NC_CACHE is None:
        _NC_CACHE = build_nc()
    return _NC_CACHE


def pack_shard(x_shard: np.ndarray) -> np.ndarray:
    """[ROWS, F] f32 -> [ROWS, 2F] u16: hi plane (bits 31..16, with bit 31
    replaced by the x<0 test) then lo plane (bits 15..0)."""
    x_shard = np.ascontiguousarray(x_shard)
    xi = x_shard.view(np.uint32)
    xi = (xi & np.uint32(0x7FFFFFFF)) | \
        ((x_shard < 0).astype(np.uint32) << np.uint32(31))
    hi = (xi >> np.uint32(16)).astype(np.uint16)
    lo = (xi & np.uint32(0xFFFF)).astype(np.uint16)
    return np.ascontiguousarray(np.concatenate([hi, lo], axis=1))


def unpack_result(out_dev: np.ndarray) -> np.ndarray:
    """Device [ROWS*NPAIR, F] u16 -> [ROWS, F, K] f32 in channel order."""
    raw = out_dev.reshape(ROWS, NPAIR, F).view(np.uint8) \
        .reshape(ROWS, NPAIR, F, 2)
    byte_k = raw.transpose(0, 2, 1, 3).reshape(ROWS, F, K)
    return byte_k[:, :, PERM].astype(np.float32)


def kernel(x: np.ndarray) -> np.ndarray:
    from concourse.bass_utils import run_bass_kernel_spmd

    x = np.asarray(x, dtype=np.float32)
    assert x.shape == (ROWS_TOTAL, F), x.shape
    nc = _get_nc()
    in_maps = [
        {"xm": pack_shard(x[i * ROWS:(i + 1) * ROWS])} for i in range(N_CORES)
    ]
    res = run_bass_kernel_spmd(nc, in_maps, list(range(N_CORES)))
    parts = [unpack_result(res.results[i]["out"]) for i in range(N_CORES)]
    return np.concatenate(parts, axis=0)
